# revision 21
# baseline (speedup 1.0000x reference)
"""Multi-head attention (B=4, S=2048, D=1024, H=16, causal) on 8 TRN2 cores.

Sharding: core c handles batch b=c//2 and head-group g=c%2 (8 heads). Each
core computes its q/k/v projections, causal attention, and a partial output
projection over all 1024 output dims; the host sums the two partial outputs
per batch and adds bo.

Design (vs. the 335us baseline):
- q/k projections run as fp8e4m3 DoubleRow matmuls (weights x16-scaled, x in
  fp8): contraction 256/instruction = 2x bf16 throughput. v/out projections,
  scores, P and v stay bf16: fp8 there fails the early causal rows where
  quantization noise does not average out.
- scores stay transposed (scT[k,q]) with the two heads of a pair on the two
  64-row halves of the split PE array (concurrent matmul pairs).
- EXP runs per k-tile ([128, 2 heads x 512] = 1024 cols/instruction) on
  double-buffered 2-bank score psums, keeping the scalar engine's EXP
  pipeline gap-free (scalar-engine fixed cost is ~260ns/instr).
- A.V is non-transposed: av[q,65] += P_tile.T @ [v+bv | 1] (M=128, N=65,
  weight loads overlap streaming). Rowsums land in column 65 per q-partition:
  normalization is a strided reciprocal + one broadcast multiply per
  (head, chunk). Accumulation groups are packed 4-per-bank sequentially
  (interleaved groups are broken on HW: start arms replace-on-write for the
  whole 2KB region; sequential complete groups + reads are safe).
- O tiles are PE-transposed ([q,d] -> [d,q]) to feed the out-projection.
- Emission interleaves projection / transpose / out-projection work between
  attention steps from filler queues so the PE never idles while the scalar
  engine streams EXPs (p-state stays at full clock).
"""

import numpy as np

B, S, D, H, Dh = 4, 2048, 1024, 16, 64
NCORES = 8
NKT = S // 128  # 16

_CACHE = {}


def _build_nc():
    import concourse.bacc as bacc
    import concourse.mybir as mybir
    import concourse.tile as tile
    from contextlib import ExitStack
    from collections import deque

    F32 = mybir.dt.float32
    BF16 = mybir.dt.bfloat16
    F8 = mybir.dt.float8e4
    DR = mybir.MatmulPerfMode.DoubleRow
    ExpF = mybir.ActivationFunctionType.Exp

    nc = bacc.Bacc("TRN2", target_bir_lowering=False, debug=False,
                   num_devices=NCORES)

    xw_d = nc.declare_dram_parameter("xw", [4, 128, 4096], BF16, isOutput=False)
    xw8_d = nc.declare_dram_parameter("xw8", [4, 128, 4096], F8, isOutput=False)
    wq8_d = nc.declare_dram_parameter("wq8", [128, 4096], F8, isOutput=False)
    wk8_d = nc.declare_dram_parameter("wk8", [128, 4096], F8, isOutput=False)
    wv_d = nc.declare_dram_parameter("wv", [128, 4096], BF16, isOutput=False)
    wo_d = nc.declare_dram_parameter("wo", [128, 4096], BF16, isOutput=False)
    bq_d = nc.declare_dram_parameter("bq16", [128, 4], F32, isOutput=False)
    bk_d = nc.declare_dram_parameter("bk16", [128, 4], F32, isOutput=False)
    bvb_d = nc.declare_dram_parameter("bvb", [128, 512], BF16, isOutput=False)
    tm_d = nc.declare_dram_parameter("trimask", [128, 128], BF16, isOutput=False)
    id_d = nc.declare_dram_parameter("idm", [128, 128], BF16, isOutput=False)
    out_d = nc.declare_dram_parameter("out", [S, D], F32, isOutput=True)

    with tile.TileContext(nc) as tc, ExitStack() as ctx:
        cpool = ctx.enter_context(tc.tile_pool(name="consts", bufs=1))
        bigpool = ctx.enter_context(tc.tile_pool(name="big", bufs=1))
        qpool = ctx.enter_context(tc.tile_pool(name="qp", bufs=8))
        ppool = ctx.enter_context(tc.tile_pool(name="pp", bufs=26))
        ospool = ctx.enter_context(tc.tile_pool(name="osp", bufs=6))
        otpool = ctx.enter_context(tc.tile_pool(name="otp", bufs=20))
        rpool = ctx.enter_context(tc.tile_pool(name="rp", bufs=4))
        mpool = ctx.enter_context(tc.tile_pool(name="mp", bufs=6))
        projps = ctx.enter_context(tc.tile_pool(name="pjps", bufs=2, space="PSUM"))
        scps = ctx.enter_context(tc.tile_pool(name="scps", bufs=2, space="PSUM"))
        avps = ctx.enter_context(tc.tile_pool(name="avps", bufs=2, space="PSUM"))

        wq8_t = cpool.tile([128, 4096], F8, name="wq8_t")
        wk8_t = cpool.tile([128, 4096], F8, name="wk8_t")
        wv_t = cpool.tile([128, 4096], BF16, name="wv_t")
        wo_t = cpool.tile([128, 4096], BF16, name="wo_t")
        bq_t = cpool.tile([128, 4], F32, name="bq_t")
        bk_t = cpool.tile([128, 4], F32, name="bk_t")
        bvb_t = cpool.tile([128, 512], BF16, name="bvb_t")
        tm_t = cpool.tile([128, 128], BF16, name="tm_t")
        id_t = cpool.tile([128, 128], BF16, name="id_t")
        K_t = bigpool.tile([128, 4 * S], BF16, name="K_t")
        vb_t = bigpool.tile([128, NKT * 520], BF16, name="vb_t")
        XW = [bigpool.tile([128, 4096], BF16, name=f"xwt{j}") for j in range(4)]
        XW8 = [bigpool.tile([128, 4096], F8, name=f"xw8t{j}") for j in range(4)]

        nc.sync.dma_start(wq8_t[:], wq8_d[:])
        nc.sync.dma_start(XW8[0][:], xw8_d[0])
        nc.sync.dma_start(wk8_t[:], wk8_d[:])
        nc.sync.dma_start(bq_t[:], bq_d[:])
        nc.sync.dma_start(bk_t[:], bk_d[:])
        nc.sync.dma_start(wv_t[:], wv_d[:])
        nc.sync.dma_start(XW[0][:], xw_d[0])
        nc.sync.dma_start(bvb_t[:], bvb_d[:])
        nc.sync.dma_start(tm_t[:], tm_d[:])
        nc.sync.dma_start(id_t[:], id_d[:])
        for j in (1, 2, 3):
            nc.sync.dma_start(XW8[j][:], xw8_d[j])
        for j in (1, 2, 3):
            nc.sync.dma_start(XW[j][:], xw_d[j])
        nc.sync.dma_start(wo_t[:], wo_d[:])

        QT = {}
        OS = {}
        OT = {}
        PT = {}

        def emit_qk_psum(pr, j, w_t, b_t, dest_ap, nm):
            # chunk 0 runs before any attention: the scores banks are idle,
            # borrow them so projection psum turnover never stalls the PE
            pool, tg = (scps, "sc") if j == 0 else (projps, "mm")
            ps = pool.tile([128, 512], F32, name=nm, tag=tg)
            for cp in range(4):
                nc.tensor.matmul(
                    ps[:],
                    w_t[:, pr * 1024 + cp * 256: pr * 1024 + (cp + 1) * 256]
                    .rearrange("p (two m) -> p two m", two=2),
                    XW8[j][:, cp * 1024: (cp + 1) * 1024]
                    .rearrange("p (two n) -> p two n", two=2),
                    start=(cp == 0), stop=(cp == 3), perf_mode=DR)
            nc.vector.tensor_scalar_add(dest_ap, ps[:], b_t[:, pr: pr + 1])

        def emit_q(pr, j):
            qt = qpool.tile([128, 512], BF16, name=f"q{pr}_{j}", tag="q")
            QT[(pr, j)] = qt
            emit_qk_psum(pr, j, wq8_t, bq_t, qt[:], f"qp{pr}_{j}")

        def emit_k(pr, j):
            emit_qk_psum(pr, j, wk8_t, bk_t,
                         K_t[:, pr * S + j * 512: pr * S + (j + 1) * 512],
                         f"kp{pr}_{j}")

        def emit_v(j, st):
            kt = 4 * j + st
            pool, tg = (scps, "sc") if j == 0 else (projps, "mm")
            ps = pool.tile([128, 512], F32, name=f"vp{kt}", tag=tg)
            for ci in range(8):
                nc.tensor.matmul(
                    ps[:],
                    XW[j][:, ci * 512 + st * 128: ci * 512 + st * 128 + 128],
                    wv_t[:, ci * 512: (ci + 1) * 512],
                    start=(ci == 0), stop=(ci == 7))
            vslot = vb_t[:, kt * 520: (kt + 1) * 520].rearrange(
                "p (h e) -> p h e", h=8)
            nc.vector.tensor_add(
                vslot[:, :, 0:64],
                ps[:].rearrange("p (h e) -> p h e", h=8),
                bvb_t[:].rearrange("p (h e) -> p h e", h=8))
            nc.gpsimd.memset(vslot[:, :, 64:65], 1.0)

        def emit_sc_one(pr, J, kt):
            off = max(0, 128 * (kt - 4 * J))
            sc = scps.tile([128, 1024], F32, name=f"sc{pr}_{J}_{kt}", tag="sc")
            scv = sc[:].rearrange("p (h q) -> p h q", h=2)
            nc.tensor.matmul(
                scv[:, 0:1, off:512],
                K_t[0:64, pr * S + kt * 128: pr * S + kt * 128 + 128],
                QT[(pr, J)][0:64, off:512], start=True, stop=True)
            nc.tensor.matmul(
                scv[:, 1:2, off:512],
                K_t[64:128, pr * S + kt * 128: pr * S + kt * 128 + 128],
                QT[(pr, J)][64:128, off:512], start=True, stop=True)
            pt = ppool.tile([128, 1024], BF16, name=f"P{pr}_{J}_{kt}", tag="p")
            PT[(pr, J, kt)] = pt
            pv = pt[:].rearrange("p (h q) -> p h q", h=2)
            nc.scalar.activation(pv[:, :, off:512], scv[:, :, off:512],
                                 ExpF, bias=0.0, scale=float(2 ** -11))
            rr = kt - 4 * J
            if rr >= 0:
                tmb = tm_t[:].rearrange("p (x q) -> p x q", x=1).broadcast_to(
                    [128, 2, 128])
                blk = pv[:, :, rr * 128: rr * 128 + 128]
                nc.vector.tensor_mul(blk, blk, tmb)

        def av_unit(pr, J, hh, qbl, av_tile):
            last = 4 * J + qbl

            def emit():
                for kt in range(last + 1):
                    pv = PT[(pr, J, kt)][:].rearrange(
                        "p (h q) -> p h q", h=2)
                    nc.tensor.matmul(
                        av_tile[:, qbl * 65: qbl * 65 + 65],
                        pv[:, hh: hh + 1,
                           qbl * 128: (qbl + 1) * 128],
                        vb_t[:, kt * 520 + (2 * pr + hh) * 65:
                             kt * 520 + (2 * pr + hh) * 65 + 65],
                        start=(kt == 0), stop=(kt == last),
                        skip_group_check=True)
            return emit

        def emit_norm(pr, J, avA, avB):
            osb = ospool.tile([128, 4, 128], BF16, name=f"os{pr}_{J}", tag="os")
            OS[(pr, J)] = osb
            for hh, av in ((0, avA), (1, avB)):
                avv = av[:].rearrange("p (qb e) -> p qb e", qb=4)
                rc = mpool.tile([128, 4, 1], F32, name=f"rc{pr}_{J}_{hh}",
                                tag="rc")
                nc.vector.reciprocal(rc[:], avv[:, :, 64:65])
                nc.vector.tensor_mul(
                    osb[:, :, hh * 64: (hh + 1) * 64],
                    avv[:, :, 0:64],
                    rc[:].broadcast_to([128, 4, 64]))

        def transp_unit(pr, J):
            def emit():
                tp_ps = avps.tile([128, 4, 128], BF16, name=f"tps{pr}_{J}",
                                  tag="av")
                osb = OS[(pr, J)]
                for qbl in range(4):
                    nc.tensor.matmul(tp_ps[:, qbl, :], osb[:, qbl, :], id_t[:],
                                     is_transpose=True, start=True, stop=True,
                                     skip_group_check=True)
                for qbl in range(4):
                    ot = otpool.tile([128, 128], BF16,
                                     name=f"otb{pr}_{J}_{qbl}", tag="ot")
                    OT[(pr, J, qbl)] = ot
                    nc.vector.tensor_copy(ot[:], tp_ps[:, qbl, :])
            return emit

        def outproj_unit(J, qbl, dmh):
            def emit():
                ps = projps.tile([128, 512], F32, name=f"op{J}_{qbl}_{dmh}",
                                 tag="mm")
                for pr in range(4):
                    nc.tensor.matmul(
                        ps[:], OT[(pr, J, qbl)][:],
                        wo_t[:, pr * 1024 + dmh * 512:
                             pr * 1024 + (dmh + 1) * 512],
                        start=(pr == 0), stop=(pr == 3))
                res = rpool.tile([128, 512], F32, name=f"res{J}_{qbl}_{dmh}",
                                 tag="res")
                nc.vector.tensor_copy(res[:], ps[:])
                qb = 4 * J + qbl
                nc.sync.dma_start(
                    out_d[qb * 128: (qb + 1) * 128,
                          dmh * 512: (dmh + 1) * 512], res[:])
            return emit

        av_q = deque()
        misc_q = deque()
        done = set()

        def drain_av(n):
            for _ in range(min(n, len(av_q))):
                av_q.popleft()()

        def drain_misc(n):
            for _ in range(min(n, len(misc_q))):
                lab, fn = misc_q.popleft()
                fn()
                done.add(lab)

        def force_until(lab):
            while lab not in done and misc_q:
                l, fn = misc_q.popleft()
                fn()
                done.add(l)

        def queue_chunk(j):
            misc_q.append((("q", 0, j), lambda: emit_q(0, j)))
            misc_q.append((("k", 0, j), lambda: emit_k(0, j)))
            for st in range(4):
                misc_q.append((("v", j, st), lambda st=st: emit_v(j, st)))
            for pr in (1, 2, 3):
                misc_q.append((("q", pr, j), lambda pr=pr: emit_q(pr, j)))
                misc_q.append((("k", pr, j), lambda pr=pr: emit_k(pr, j)))

        queue_chunk(0)
        queue_chunk(1)

        prev = None
        for J in range(4):
            if J >= 1 and J + 1 <= 3:
                queue_chunk(J + 1)
            for pr in range(4):
                force_until(("k", pr, J))
                for kt in range(4 * J + 4):
                    emit_sc_one(pr, J, kt)
                    drain_av(1)
                    if kt % 2 == 1:
                        drain_misc(1)
                while av_q:
                    av_q.popleft()()
                if prev is not None:
                    pJ, ppr, pavA, pavB = prev
                    emit_norm(ppr, pJ, pavA, pavB)
                    misc_q.append((("t", ppr, pJ), transp_unit(ppr, pJ)))
                    if ppr == 3:
                        for qbl in range(4):
                            for dmh in range(2):
                                misc_q.append(
                                    (("o", pJ, qbl, dmh),
                                     outproj_unit(pJ, qbl, dmh)))
                avA = avps.tile([128, 260], F32, name=f"avA{pr}_{J}", tag="av")
                avB = avps.tile([128, 260], F32, name=f"avB{pr}_{J}", tag="av")
                for qbl in range(4):
                    av_q.append(av_unit(pr, J, 0, qbl, avA))
                    av_q.append(av_unit(pr, J, 1, qbl, avB))
                prev = (J, pr, avA, avB)
        # epilogue: finish remaining misc (transposes/outproj of earlier prs)
        while misc_q:
            lab, fn = misc_q.popleft()
            fn()
        # last pr (J=3, pr=3): per-qblock pipeline av -> norm -> transpose ->
        # outproj so the output projection overlaps the remaining attention.
        pJ, ppr, pavA, pavB = prev
        osb = ospool.tile([128, 4, 128], BF16, name=f"os{ppr}_{pJ}", tag="os")
        OS[(ppr, pJ)] = osb
        tp_ps = avps.tile([128, 4, 128], BF16, name=f"tps{ppr}_{pJ}", tag="av")
        for qbl in range(4):
            for hh, av in ((0, pavA), (1, pavB)):
                av_q.popleft()()  # av_unit(ppr, pJ, hh, qbl)
            for hh, av in ((0, pavA), (1, pavB)):
                avv = av[:].rearrange("p (qb e) -> p qb e", qb=4)
                rc = mpool.tile([128, 1, 1], F32, name=f"rcl{qbl}_{hh}",
                                tag="rc")
                nc.vector.reciprocal(rc[:], avv[:, qbl: qbl + 1, 64:65])
                nc.vector.tensor_mul(
                    osb[:, qbl: qbl + 1, hh * 64: (hh + 1) * 64],
                    avv[:, qbl: qbl + 1, 0:64],
                    rc[:].broadcast_to([128, 1, 64]))
            nc.tensor.matmul(tp_ps[:, qbl, :], osb[:, qbl, :], id_t[:],
                             is_transpose=True, start=True, stop=True,
                             skip_group_check=True)
            ot = otpool.tile([128, 128], BF16, name=f"otb{ppr}_{pJ}_{qbl}",
                             tag="ot")
            OT[(ppr, pJ, qbl)] = ot
            nc.vector.tensor_copy(ot[:], tp_ps[:, qbl, :])
            for dmh in range(2):
                outproj_unit(3, qbl, dmh)()

    nc.compile()
    return nc


def _get_nc():
    if "nc" not in _CACHE:
        _CACHE["nc"] = _build_nc()
    return _CACHE["nc"]


def make_in_maps(x, mask, Wq, bq, Wk, bk, Wv, bv, Wo, bo):
    import ml_dtypes
    f32 = np.float32
    bf16 = ml_dtypes.bfloat16
    f8 = ml_dtypes.float8_e4m3fn
    assert np.all(np.asarray(mask) == 1), "kernel assumes all-ones pad mask"
    tm = np.triu(np.ones((128, 128), f32)).astype(bf16)
    idm = np.eye(128, dtype=f32).astype(bf16)
    in_maps = []
    for c in range(NCORES):
        b, g = c // 2, c % 2
        sl = slice(g * 512, (g + 1) * 512)
        xb = np.asarray(x[b], f32)
        xw = np.ascontiguousarray(
            xb.reshape(4, 512, 8, 128).transpose(0, 3, 2, 1)
            .reshape(4, 128, 4096))

        def wlay8(W):
            Ws = np.asarray(W, f32)[sl] * 16.0
            return np.ascontiguousarray(
                Ws.reshape(4, 128, 4, 2, 128).transpose(4, 0, 2, 3, 1)
                .reshape(128, 4096)).astype(f8)

        wo = np.ascontiguousarray(
            np.asarray(Wo, f32)[:, sl].T.reshape(4, 128, 1024)
            .transpose(1, 0, 2).reshape(128, 4096)).astype(bf16)
        wv = np.ascontiguousarray(
            np.asarray(Wv, f32)[sl].reshape(512, 8, 128).transpose(2, 1, 0)
            .reshape(128, 4096)).astype(bf16)
        in_maps.append({
            "xw": xw.astype(bf16), "xw8": xw.astype(f8),
            "wq8": wlay8(Wq), "wk8": wlay8(Wk), "wv": wv, "wo": wo,
            "bq16": np.ascontiguousarray(
                (16.0 * np.asarray(bq, f32)[sl]).reshape(4, 128).T),
            "bk16": np.ascontiguousarray(
                (16.0 * np.asarray(bk, f32)[sl]).reshape(4, 128).T),
            "bvb": np.tile(np.asarray(bv, f32)[sl], (128, 1)).astype(bf16),
            "trimask": tm, "idm": idm,
        })
    return in_maps


def kernel(x, mask, Wq, bq, Wk, bk, Wv, bv, Wo, bo):
    from concourse.bass_utils import run_bass_kernel_spmd

    nc = _get_nc()
    in_maps = make_in_maps(x, mask, Wq, bq, Wk, bk, Wv, bv, Wo, bo)
    res = run_bass_kernel_spmd(nc, in_maps, list(range(NCORES))).results
    out = np.empty((B, S, D), np.float32)
    bo32 = np.asarray(bo, np.float32)
    for b in range(B):
        out[b] = res[2 * b]["out"] + res[2 * b + 1]["out"] + bo32
    return out


# revision 22
# speedup vs baseline: 1.0134x; 1.0134x over previous
"""Multi-head attention (B=4, S=2048, D=1024, H=16, causal) on 8 TRN2 cores.

Sharding: core c handles batch b=c//2 and head-group g=c%2 (8 heads). Each
core computes its q/k/v projections, causal attention, and a partial output
projection over all 1024 output dims; the host sums the two partial outputs
per batch and adds bo.

Design (vs. the 335us baseline):
- q/k projections run as fp8e4m3 DoubleRow matmuls (weights x16-scaled, x in
  fp8): contraction 256/instruction = 2x bf16 throughput. v/out projections,
  scores, P and v stay bf16: fp8 there fails the early causal rows where
  quantization noise does not average out.
- scores stay transposed (scT[k,q]) with the two heads of a pair on the two
  64-row halves of the split PE array (concurrent matmul pairs).
- EXP runs per k-tile ([128, 2 heads x 512] = 1024 cols/instruction) on
  double-buffered 2-bank score psums, keeping the scalar engine's EXP
  pipeline gap-free (scalar-engine fixed cost is ~260ns/instr).
- A.V is non-transposed: av[q,65] += P_tile.T @ [v+bv | 1] (M=128, N=65,
  weight loads overlap streaming). Rowsums land in column 65 per q-partition:
  normalization is a strided reciprocal + one broadcast multiply per
  (head, chunk). Accumulation groups are packed 4-per-bank sequentially
  (interleaved groups are broken on HW: start arms replace-on-write for the
  whole 2KB region; sequential complete groups + reads are safe).
- O tiles are PE-transposed ([q,d] -> [d,q]) to feed the out-projection.
- Emission interleaves projection / transpose / out-projection work between
  attention steps from filler queues so the PE never idles while the scalar
  engine streams EXPs (p-state stays at full clock).
"""

import numpy as np

B, S, D, H, Dh = 4, 2048, 1024, 16, 64
NCORES = 8
NKT = S // 128  # 16

_CACHE = {}


def _build_nc():
    import concourse.bacc as bacc
    import concourse.mybir as mybir
    import concourse.tile as tile
    from contextlib import ExitStack
    from collections import deque

    F32 = mybir.dt.float32
    BF16 = mybir.dt.bfloat16
    F8 = mybir.dt.float8e4
    DR = mybir.MatmulPerfMode.DoubleRow
    ExpF = mybir.ActivationFunctionType.Exp

    nc = bacc.Bacc("TRN2", target_bir_lowering=False, debug=False,
                   num_devices=NCORES)

    xw_d = nc.declare_dram_parameter("xw", [4, 128, 4096], BF16, isOutput=False)
    xw8_d = nc.declare_dram_parameter("xw8", [4, 128, 4096], F8, isOutput=False)
    wq8_d = nc.declare_dram_parameter("wq8", [128, 4096], F8, isOutput=False)
    wk8_d = nc.declare_dram_parameter("wk8", [128, 4096], F8, isOutput=False)
    wv_d = nc.declare_dram_parameter("wv", [128, 4096], BF16, isOutput=False)
    wo_d = nc.declare_dram_parameter("wo", [128, 4096], BF16, isOutput=False)
    bq_d = nc.declare_dram_parameter("bq16", [128, 4], F32, isOutput=False)
    bk_d = nc.declare_dram_parameter("bk16", [128, 4], F32, isOutput=False)
    bvb_d = nc.declare_dram_parameter("bvb", [128, 512], BF16, isOutput=False)
    tm_d = nc.declare_dram_parameter("trimask", [128, 128], BF16, isOutput=False)
    id_d = nc.declare_dram_parameter("idm", [128, 128], BF16, isOutput=False)
    out_d = nc.declare_dram_parameter("out", [S, D], F32, isOutput=True)

    with tile.TileContext(nc) as tc, ExitStack() as ctx:
        cpool = ctx.enter_context(tc.tile_pool(name="consts", bufs=1))
        bigpool = ctx.enter_context(tc.tile_pool(name="big", bufs=1))
        qpool = ctx.enter_context(tc.tile_pool(name="qp", bufs=8))
        ppool = ctx.enter_context(tc.tile_pool(name="pp", bufs=26))
        ospool = ctx.enter_context(tc.tile_pool(name="osp", bufs=6))
        otpool = ctx.enter_context(tc.tile_pool(name="otp", bufs=20))
        rpool = ctx.enter_context(tc.tile_pool(name="rp", bufs=4))
        mpool = ctx.enter_context(tc.tile_pool(name="mp", bufs=6))
        projps = ctx.enter_context(tc.tile_pool(name="pjps", bufs=2, space="PSUM"))
        scps = ctx.enter_context(tc.tile_pool(name="scps", bufs=2, space="PSUM"))
        avps = ctx.enter_context(tc.tile_pool(name="avps", bufs=2, space="PSUM"))

        wq8_t = cpool.tile([128, 4096], F8, name="wq8_t")
        wk8_t = cpool.tile([128, 4096], F8, name="wk8_t")
        wv_t = cpool.tile([128, 4096], BF16, name="wv_t")
        wo_t = cpool.tile([128, 4096], BF16, name="wo_t")
        bq_t = cpool.tile([128, 4], F32, name="bq_t")
        bk_t = cpool.tile([128, 4], F32, name="bk_t")
        bvb_t = cpool.tile([128, 512], BF16, name="bvb_t")
        tm_t = cpool.tile([128, 128], BF16, name="tm_t")
        id_t = cpool.tile([128, 128], BF16, name="id_t")
        K_t = bigpool.tile([128, 4 * S], BF16, name="K_t")
        vb_t = bigpool.tile([128, NKT * 520], BF16, name="vb_t")
        XW = [bigpool.tile([128, 4096], BF16, name=f"xwt{j}") for j in range(4)]
        XW8 = [bigpool.tile([128, 4096], F8, name=f"xw8t{j}") for j in range(4)]

        nc.sync.dma_start(wq8_t[:], wq8_d[:])
        nc.sync.dma_start(XW8[0][:], xw8_d[0])
        nc.sync.dma_start(wk8_t[:], wk8_d[:])
        nc.sync.dma_start(bq_t[:], bq_d[:])
        nc.sync.dma_start(bk_t[:], bk_d[:])
        nc.sync.dma_start(wv_t[:], wv_d[:])
        nc.sync.dma_start(XW[0][:], xw_d[0])
        nc.sync.dma_start(bvb_t[:], bvb_d[:])
        nc.sync.dma_start(tm_t[:], tm_d[:])
        nc.sync.dma_start(id_t[:], id_d[:])
        for j in (1, 2, 3):
            nc.sync.dma_start(XW8[j][:], xw8_d[j])
        for j in (1, 2, 3):
            nc.sync.dma_start(XW[j][:], xw_d[j])
        nc.sync.dma_start(wo_t[:], wo_d[:])

        QT = {}
        OS = {}
        OT = {}
        PT = {}

        def emit_qk_psum(pr, j, w_t, b_t, dest_ap, nm):
            ps = projps.tile([128, 512], F32, name=nm, tag="mm")
            for cp in range(4):
                nc.tensor.matmul(
                    ps[:],
                    w_t[:, pr * 1024 + cp * 256: pr * 1024 + (cp + 1) * 256]
                    .rearrange("p (two m) -> p two m", two=2),
                    XW8[j][:, cp * 1024: (cp + 1) * 1024]
                    .rearrange("p (two n) -> p two n", two=2),
                    start=(cp == 0), stop=(cp == 3), perf_mode=DR)
            nc.vector.tensor_scalar_add(dest_ap, ps[:], b_t[:, pr: pr + 1])

        def emit_q(pr, j):
            qt = qpool.tile([128, 512], BF16, name=f"q{pr}_{j}", tag="q")
            QT[(pr, j)] = qt
            emit_qk_psum(pr, j, wq8_t, bq_t, qt[:], f"qp{pr}_{j}")

        def emit_k(pr, j):
            emit_qk_psum(pr, j, wk8_t, bk_t,
                         K_t[:, pr * S + j * 512: pr * S + (j + 1) * 512],
                         f"kp{pr}_{j}")

        def emit_v(j, st):
            kt = 4 * j + st
            ps = projps.tile([128, 512], F32, name=f"vp{kt}", tag="mm")
            for ci in range(8):
                nc.tensor.matmul(
                    ps[:],
                    XW[j][:, ci * 512 + st * 128: ci * 512 + st * 128 + 128],
                    wv_t[:, ci * 512: (ci + 1) * 512],
                    start=(ci == 0), stop=(ci == 7))
            vslot = vb_t[:, kt * 520: (kt + 1) * 520].rearrange(
                "p (h e) -> p h e", h=8)
            nc.vector.tensor_add(
                vslot[:, :, 0:64],
                ps[:].rearrange("p (h e) -> p h e", h=8),
                bvb_t[:].rearrange("p (h e) -> p h e", h=8))
            nc.gpsimd.memset(vslot[:, :, 64:65], 1.0)

        def emit_sc_one(pr, J, kt):
            off = max(0, 128 * (kt - 4 * J))
            sc = scps.tile([128, 1024], F32, name=f"sc{pr}_{J}_{kt}", tag="sc")
            scv = sc[:].rearrange("p (h q) -> p h q", h=2)
            nc.tensor.matmul(
                scv[:, 0:1, off:512],
                K_t[0:64, pr * S + kt * 128: pr * S + kt * 128 + 128],
                QT[(pr, J)][0:64, off:512], start=True, stop=True)
            nc.tensor.matmul(
                scv[:, 1:2, off:512],
                K_t[64:128, pr * S + kt * 128: pr * S + kt * 128 + 128],
                QT[(pr, J)][64:128, off:512], start=True, stop=True)
            pt = ppool.tile([128, 1024], BF16, name=f"P{pr}_{J}_{kt}", tag="p")
            PT[(pr, J, kt)] = pt
            pv = pt[:].rearrange("p (h q) -> p h q", h=2)
            nc.scalar.activation(pv[:, :, off:512], scv[:, :, off:512],
                                 ExpF, bias=0.0, scale=float(2 ** -11))
            rr = kt - 4 * J
            if rr >= 0:
                tmb = tm_t[:].rearrange("p (x q) -> p x q", x=1).broadcast_to(
                    [128, 2, 128])
                blk = pv[:, :, rr * 128: rr * 128 + 128]
                nc.vector.tensor_mul(blk, blk, tmb)

        def av_unit(pr, J, hh, qbl, av_tile):
            last = 4 * J + qbl

            def emit():
                for kt in range(last + 1):
                    pv = PT[(pr, J, kt)][:].rearrange(
                        "p (h q) -> p h q", h=2)
                    nc.tensor.matmul(
                        av_tile[:, qbl * 65: qbl * 65 + 65],
                        pv[:, hh: hh + 1,
                           qbl * 128: (qbl + 1) * 128],
                        vb_t[:, kt * 520 + (2 * pr + hh) * 65:
                             kt * 520 + (2 * pr + hh) * 65 + 65],
                        start=(kt == 0), stop=(kt == last),
                        skip_group_check=True)
            return emit

        def emit_norm(pr, J, avA, avB):
            osb = ospool.tile([128, 4, 128], BF16, name=f"os{pr}_{J}", tag="os")
            OS[(pr, J)] = osb
            for hh, av in ((0, avA), (1, avB)):
                avv = av[:].rearrange("p (qb e) -> p qb e", qb=4)
                rc = mpool.tile([128, 4, 1], F32, name=f"rc{pr}_{J}_{hh}",
                                tag="rc")
                nc.vector.reciprocal(rc[:], avv[:, :, 64:65])
                nc.vector.tensor_mul(
                    osb[:, :, hh * 64: (hh + 1) * 64],
                    avv[:, :, 0:64],
                    rc[:].broadcast_to([128, 4, 64]))

        def transp_unit(pr, J):
            def emit():
                tp_ps = avps.tile([128, 4, 128], BF16, name=f"tps{pr}_{J}",
                                  tag="av")
                osb = OS[(pr, J)]
                for qbl in range(4):
                    nc.tensor.matmul(tp_ps[:, qbl, :], osb[:, qbl, :], id_t[:],
                                     is_transpose=True, start=True, stop=True,
                                     skip_group_check=True)
                for qbl in range(4):
                    ot = otpool.tile([128, 128], BF16,
                                     name=f"otb{pr}_{J}_{qbl}", tag="ot")
                    OT[(pr, J, qbl)] = ot
                    nc.vector.tensor_copy(ot[:], tp_ps[:, qbl, :])
            return emit

        def outproj_unit(J, qbl, dmh):
            def emit():
                ps = projps.tile([128, 512], F32, name=f"op{J}_{qbl}_{dmh}",
                                 tag="mm")
                for pr in range(4):
                    nc.tensor.matmul(
                        ps[:], OT[(pr, J, qbl)][:],
                        wo_t[:, pr * 1024 + dmh * 512:
                             pr * 1024 + (dmh + 1) * 512],
                        start=(pr == 0), stop=(pr == 3))
                res = rpool.tile([128, 512], F32, name=f"res{J}_{qbl}_{dmh}",
                                 tag="res")
                nc.vector.tensor_copy(res[:], ps[:])
                qb = 4 * J + qbl
                nc.sync.dma_start(
                    out_d[qb * 128: (qb + 1) * 128,
                          dmh * 512: (dmh + 1) * 512], res[:])
            return emit

        av_q = deque()
        misc_q = deque()
        done = set()

        def drain_av(n):
            for _ in range(min(n, len(av_q))):
                av_q.popleft()()

        def drain_misc(n):
            for _ in range(min(n, len(misc_q))):
                lab, fn = misc_q.popleft()
                fn()
                done.add(lab)

        def force_until(lab):
            while lab not in done and misc_q:
                l, fn = misc_q.popleft()
                fn()
                done.add(l)

        def queue_chunk(j):
            misc_q.append((("q", 0, j), lambda: emit_q(0, j)))
            misc_q.append((("k", 0, j), lambda: emit_k(0, j)))
            for st in range(4):
                misc_q.append((("v", j, st), lambda st=st: emit_v(j, st)))
            for pr in (1, 2, 3):
                misc_q.append((("q", pr, j), lambda pr=pr: emit_q(pr, j)))
                misc_q.append((("k", pr, j), lambda pr=pr: emit_k(pr, j)))

        queue_chunk(0)
        queue_chunk(1)

        prev = None
        for J in range(4):
            if J >= 1 and J + 1 <= 3:
                queue_chunk(J + 1)
            for pr in range(4):
                force_until(("k", pr, J))
                for kt in range(4 * J + 4):
                    emit_sc_one(pr, J, kt)
                    drain_av(1)
                    if kt % 2 == 1:
                        drain_misc(1)
                while av_q:
                    av_q.popleft()()
                if prev is not None:
                    pJ, ppr, pavA, pavB = prev
                    emit_norm(ppr, pJ, pavA, pavB)
                    misc_q.append((("t", ppr, pJ), transp_unit(ppr, pJ)))
                    if ppr == 3:
                        for qbl in range(4):
                            for dmh in range(2):
                                misc_q.append(
                                    (("o", pJ, qbl, dmh),
                                     outproj_unit(pJ, qbl, dmh)))
                avA = avps.tile([128, 260], F32, name=f"avA{pr}_{J}", tag="av")
                avB = avps.tile([128, 260], F32, name=f"avB{pr}_{J}", tag="av")
                for qbl in range(4):
                    av_q.append(av_unit(pr, J, 0, qbl, avA))
                    av_q.append(av_unit(pr, J, 1, qbl, avB))
                prev = (J, pr, avA, avB)
        # epilogue: finish remaining misc (transposes/outproj of earlier prs)
        while misc_q:
            lab, fn = misc_q.popleft()
            fn()
        # last pr (J=3, pr=3): per-qblock pipeline av -> norm -> transpose ->
        # outproj so the output projection overlaps the remaining attention.
        pJ, ppr, pavA, pavB = prev
        osb = ospool.tile([128, 4, 128], BF16, name=f"os{ppr}_{pJ}", tag="os")
        OS[(ppr, pJ)] = osb
        tp_ps = avps.tile([128, 4, 128], BF16, name=f"tps{ppr}_{pJ}", tag="av")
        for qbl in range(4):
            for hh, av in ((0, pavA), (1, pavB)):
                av_q.popleft()()  # av_unit(ppr, pJ, hh, qbl)
            for hh, av in ((0, pavA), (1, pavB)):
                avv = av[:].rearrange("p (qb e) -> p qb e", qb=4)
                rc = mpool.tile([128, 1, 1], F32, name=f"rcl{qbl}_{hh}",
                                tag="rc")
                nc.vector.reciprocal(rc[:], avv[:, qbl: qbl + 1, 64:65])
                nc.vector.tensor_mul(
                    osb[:, qbl: qbl + 1, hh * 64: (hh + 1) * 64],
                    avv[:, qbl: qbl + 1, 0:64],
                    rc[:].broadcast_to([128, 1, 64]))
            nc.tensor.matmul(tp_ps[:, qbl, :], osb[:, qbl, :], id_t[:],
                             is_transpose=True, start=True, stop=True,
                             skip_group_check=True)
            ot = otpool.tile([128, 128], BF16, name=f"otb{ppr}_{pJ}_{qbl}",
                             tag="ot")
            OT[(ppr, pJ, qbl)] = ot
            nc.vector.tensor_copy(ot[:], tp_ps[:, qbl, :])
            for dmh in range(2):
                outproj_unit(3, qbl, dmh)()

    nc.compile()
    return nc


def _get_nc():
    if "nc" not in _CACHE:
        _CACHE["nc"] = _build_nc()
    return _CACHE["nc"]


def make_in_maps(x, mask, Wq, bq, Wk, bk, Wv, bv, Wo, bo):
    import ml_dtypes
    f32 = np.float32
    bf16 = ml_dtypes.bfloat16
    f8 = ml_dtypes.float8_e4m3fn
    assert np.all(np.asarray(mask) == 1), "kernel assumes all-ones pad mask"
    tm = np.triu(np.ones((128, 128), f32)).astype(bf16)
    idm = np.eye(128, dtype=f32).astype(bf16)
    in_maps = []
    for c in range(NCORES):
        b, g = c // 2, c % 2
        sl = slice(g * 512, (g + 1) * 512)
        xb = np.asarray(x[b], f32)
        xw = np.ascontiguousarray(
            xb.reshape(4, 512, 8, 128).transpose(0, 3, 2, 1)
            .reshape(4, 128, 4096))

        def wlay8(W):
            Ws = np.asarray(W, f32)[sl] * 16.0
            return np.ascontiguousarray(
                Ws.reshape(4, 128, 4, 2, 128).transpose(4, 0, 2, 3, 1)
                .reshape(128, 4096)).astype(f8)

        wo = np.ascontiguousarray(
            np.asarray(Wo, f32)[:, sl].T.reshape(4, 128, 1024)
            .transpose(1, 0, 2).reshape(128, 4096)).astype(bf16)
        wv = np.ascontiguousarray(
            np.asarray(Wv, f32)[sl].reshape(512, 8, 128).transpose(2, 1, 0)
            .reshape(128, 4096)).astype(bf16)
        in_maps.append({
            "xw": xw.astype(bf16), "xw8": xw.astype(f8),
            "wq8": wlay8(Wq), "wk8": wlay8(Wk), "wv": wv, "wo": wo,
            "bq16": np.ascontiguousarray(
                (16.0 * np.asarray(bq, f32)[sl]).reshape(4, 128).T),
            "bk16": np.ascontiguousarray(
                (16.0 * np.asarray(bk, f32)[sl]).reshape(4, 128).T),
            "bvb": np.tile(np.asarray(bv, f32)[sl], (128, 1)).astype(bf16),
            "trimask": tm, "idm": idm,
        })
    return in_maps


def kernel(x, mask, Wq, bq, Wk, bk, Wv, bv, Wo, bo):
    from concourse.bass_utils import run_bass_kernel_spmd

    nc = _get_nc()
    in_maps = make_in_maps(x, mask, Wq, bq, Wk, bk, Wv, bv, Wo, bo)
    res = run_bass_kernel_spmd(nc, in_maps, list(range(NCORES))).results
    out = np.empty((B, S, D), np.float32)
    bo32 = np.asarray(bo, np.float32)
    for b in range(B):
        out[b] = res[2 * b]["out"] + res[2 * b + 1]["out"] + bo32
    return out
